# revision 3
# baseline (speedup 1.0000x reference)
"""Trainium2 Bass kernel for ClassedProjectedAdaptiveLogSoftmax (8-core SPMD).

Strategy (vocab-tensor-parallel, per the sharding hint):
  - The 100k vocab is split into head (10k) + 3 tail clusters (10k/30k/50k).
    Each of the 8 cores takes a contiguous 1/8 slice of EVERY cluster
    (1250 + 1250 + 3750 + 6250 = 12500 rows), so per-cluster exp-sums are
    partial sums that a single 32KB AllReduce(add) combines.
  - log_softmax is computed without max-subtraction (logits are ~N(0, 0.45^2),
    exp is safely in fp32 range), so per-cluster logsumexp = log(allreduce(
    sum(exp(logit) * exp(bias)))).  Root-column masking folds into exp(bias)=0.
  - The per-token target logit (the gather) is computed on every core as a
    row-wise dot h[t]·weight[target[t]] using host-gathered weight rows, and
    the 3 head cluster-column logits via a tiny matmul; the final NLL combine
    runs on-device after the AllReduce.

Device layout: tokens on partitions (p = t % 128, m = t // 128, 16 m-tiles).
Main loop: 10 "superchunks" of 1250 vocab cols (exactly cluster-aligned),
matmul (bf16) -> PSUM -> ScalarE exp -> VectorE (*exp(bias)) with fused
per-partition accumulate -> per-(m, cluster) partial sums.
"""

import numpy as np
import ml_dtypes

import concourse.bacc as bacc
import concourse.tile as tile
import concourse.bass_utils as bass_utils
from concourse import mybir

F32 = mybir.dt.float32
BF16 = mybir.dt.bfloat16
AF = mybir.ActivationFunctionType
ALU = mybir.AluOpType

N_CORES = 8
N = 2048            # tokens
D = 512             # hidden
M_TILES = N // 128  # 16
K_TILES = D // 128  # 4
CUTOFF_ENDS = [0, 10000, 20000, 50000, 100000]
CL_ROOT = [5, 17, 123, 10005, 20007, 50011]
CL_W = [1250, 1250, 3750, 6250]   # per-core width of each cluster slice
SLICE = sum(CL_W)                 # 12500
SC_W = 1250                       # superchunk width (cluster-aligned)
N_SC = SLICE // SC_W              # 10
# superchunk index ranges per cluster: head [0,1), c1 [1,2), c2 [2,5), c3 [5,10)
CL_SC = [(0, 1), (1, 2), (2, 5), (5, 10)]
SUBCH = [(0, 512), (512, 512), (1024, 226)]  # matmul N-chunks within a superchunk

_CACHE = {}


def _build(collective=True):
    nc = bacc.Bacc("TRN2", target_bir_lowering=False, debug=False,
                   enable_asserts=True, num_devices=N_CORES)

    wt = nc.dram_tensor("wt", [128, K_TILES, SLICE], BF16, kind="ExternalInput").ap()
    eb = nc.dram_tensor("eb", [128, SLICE], F32, kind="ExternalInput").ap()
    ht = nc.dram_tensor("ht", [128, K_TILES, N], BF16, kind="ExternalInput").ap()
    hb = nc.dram_tensor("hb", [128, M_TILES, D], BF16, kind="ExternalInput").ap()
    gw = nc.dram_tensor("gw", [128, M_TILES, D], BF16, kind="ExternalInput").ap()
    gb = nc.dram_tensor("gb", [128, M_TILES], F32, kind="ExternalInput").ap()
    cw = nc.dram_tensor("cw", [128, K_TILES, 3], BF16, kind="ExternalInput").ap()
    cb = nc.dram_tensor("cb", [128, 3], F32, kind="ExternalInput").ap()
    mk = nc.dram_tensor("mk", [128, M_TILES, 3], F32, kind="ExternalInput").ap()
    out = nc.dram_tensor("nll", [128, M_TILES], F32, kind="ExternalOutput").ap()

    with tile.TileContext(nc) as tc:
        with tc.tile_pool(name="const", bufs=1) as cpool, \
             tc.tile_pool(name="wp", bufs=3) as wpool, \
             tc.tile_pool(name="ep", bufs=3) as epool, \
             tc.tile_pool(name="xp", bufs=3) as xpool, \
             tc.tile_pool(name="sp", bufs=3) as spool, \
             tc.tile_pool(name="gp", bufs=2) as gpool, \
             tc.tile_pool(name="ps", bufs=2, space="PSUM") as pspool, \
             tc.tile_pool(name="ps3", bufs=2, space="PSUM") as ps3pool, \
             tc.tile_pool(name="dram", bufs=1, space="DRAM") as dpool:

            # ---- resident inputs ----
            ht_t = cpool.tile([128, K_TILES, N], BF16, tag="ht")
            nc.sync.dma_start(out=ht_t[:], in_=ht[:])
            hb_t = cpool.tile([128, M_TILES, D], BF16, tag="hb")
            nc.sync.dma_start(out=hb_t[:], in_=hb[:])
            gw_t = cpool.tile([128, M_TILES, D], BF16, tag="gw")
            nc.sync.dma_start(out=gw_t[:], in_=gw[:])
            gb_t = cpool.tile([128, M_TILES], F32, tag="gb")
            nc.sync.dma_start(out=gb_t[:], in_=gb[:])
            cw_t = cpool.tile([128, K_TILES, 3], BF16, tag="cw")
            nc.sync.dma_start(out=cw_t[:], in_=cw[:])
            cb_t = cpool.tile([128, 3], F32, tag="cb")
            nc.sync.dma_start(out=cb_t[:], in_=cb[:])
            mk_t = cpool.tile([128, M_TILES, 3], F32, tag="mk")
            nc.sync.dma_start(out=mk_t[:], in_=mk[:])

            # ---- resident workspaces ----
            part = cpool.tile([128, M_TILES * N_SC], F32, tag="part")  # [m*10+sc]
            l3 = cpool.tile([128, M_TILES * 3], F32, tag="l3")         # [m*3+j]
            e3 = cpool.tile([128, M_TILES * 3], F32, tag="e3")
            gd = cpool.tile([128, M_TILES], F32, tag="gd")
            gfull = cpool.tile([128, M_TILES], F32, tag="gfull")
            S = cpool.tile([128, 4 * M_TILES], F32, tag="S")           # [c*16+m]
            St = cpool.tile([128, 4 * M_TILES], F32, tag="St")

            # ---- main vocab loop ----
            for sc in range(N_SC):
                n0 = sc * SC_W
                wt_c = wpool.tile([128, K_TILES, SC_W], BF16)
                nc.sync.dma_start(out=wt_c[:], in_=wt[:, :, n0:n0 + SC_W])
                eb_c = epool.tile([128, SC_W], F32)
                nc.sync.dma_start(out=eb_c[:], in_=eb[:, n0:n0 + SC_W])
                for m in range(M_TILES):
                    ps = pspool.tile([128, SC_W], F32)
                    for kk in range(K_TILES):
                        lhsT = ht_t[:, kk, m * 128:(m + 1) * 128]
                        for coff, cwid in SUBCH:
                            nc.tensor.matmul(
                                ps[:, coff:coff + cwid], lhsT,
                                wt_c[:, kk, coff:coff + cwid],
                                start=(kk == 0), stop=(kk == K_TILES - 1))
                    ex = xpool.tile([128, SC_W], F32)
                    nc.scalar.activation(ex[:], ps[:], AF.Exp)
                    sco = spool.tile([128, SC_W], F32)
                    col = m * N_SC + sc
                    nc.vector.scalar_tensor_tensor(
                        out=sco[:], in0=ex[:], scalar=1.0, in1=eb_c[:],
                        op0=ALU.mult, op1=ALU.mult,
                        accum_out=part[:, col:col + 1])

            # ---- target-row dot: gd[:, m] = sum_d hb*gw ----
            for m in range(M_TILES):
                gsc = gpool.tile([128, D], F32)
                nc.vector.scalar_tensor_tensor(
                    out=gsc[:], in0=hb_t[:, m, :], scalar=1.0, in1=gw_t[:, m, :],
                    op0=ALU.mult, op1=ALU.mult,
                    accum_out=gd[:, m:m + 1])
            nc.vector.tensor_add(gfull[:], gd[:], gb_t[:])

            # ---- head cluster-column logits l3 [m*3+j] ----
            for m in range(M_TILES):
                ps3 = ps3pool.tile([128, 3], F32)
                for kk in range(K_TILES):
                    nc.tensor.matmul(
                        ps3[:], ht_t[:, kk, m * 128:(m + 1) * 128], cw_t[:, kk, :],
                        start=(kk == 0), stop=(kk == K_TILES - 1))
                nc.vector.tensor_add(l3[:, m * 3:(m + 1) * 3], ps3[:], cb_t[:])

            # ---- per-cluster partial sums S[c*16+m] ----
            for c, (s0, s1) in enumerate(CL_SC):
                for m in range(M_TILES):
                    nc.vector.tensor_reduce(
                        S[:, c * M_TILES + m:c * M_TILES + m + 1],
                        part[:, m * N_SC + s0:m * N_SC + s1],
                        axis=mybir.AxisListType.X, op=ALU.add)

            # ---- AllReduce partial sums across the 8 cores ----
            in_b = dpool.tile([128, 4 * M_TILES], F32, tag="arin")
            out_b = dpool.tile([128, 4 * M_TILES], F32, tag="arout")
            nc.sync.dma_start(out=in_b[:], in_=S[:])
            if collective:
                nc.gpsimd.collective_compute(
                    "AllReduce", ALU.add,
                    replica_groups=[list(range(N_CORES))],
                    ins=[in_b.opt()], outs=[out_b.opt()])
            else:  # timeline-sim build: collectives unsupported, plain copy
                nc.sync.dma_start(out=out_b[:], in_=in_b[:])
            nc.sync.dma_start(out=St[:], in_=out_b[:])

            # ---- combine: head sum += cluster-col exps (counted once) ----
            nc.scalar.activation(e3[:], l3[:], AF.Exp)
            h0 = cpool.tile([128, M_TILES], F32, tag="h0")
            h1 = cpool.tile([128, M_TILES], F32, tag="h1")
            h2 = cpool.tile([128, M_TILES], F32, tag="h2")
            nc.vector.tensor_add(h0[:], St[:, 0:M_TILES], e3[:, 0::3])
            nc.vector.tensor_add(h1[:], h0[:], e3[:, 1::3])
            nc.vector.tensor_add(h2[:], h1[:], e3[:, 2::3])

            # ---- logs ----
            Lh = cpool.tile([128, M_TILES], F32, tag="Lh")
            nc.scalar.activation(Lh[:], h2[:], AF.Ln)
            Lc = cpool.tile([128, 3 * M_TILES], F32, tag="Lc")
            nc.scalar.activation(Lc[:], St[:, M_TILES:4 * M_TILES], AF.Ln)

            # ---- nll = Lh - g + sum_i mk_i * (Lc_i - l3[3-i]) ----
            bacc_t = None
            for i in (1, 2, 3):
                j = 3 - i
                ti = cpool.tile([128, M_TILES], F32, tag=f"t{i}")
                nc.vector.tensor_sub(
                    ti[:], Lc[:, (i - 1) * M_TILES:i * M_TILES], l3[:, j::3])
                bi = cpool.tile([128, M_TILES], F32, tag=f"b{i}")
                nc.vector.tensor_mul(bi[:], ti[:], mk_t[:, :, i - 1])
                if bacc_t is None:
                    bacc_t = bi
                else:
                    nb = cpool.tile([128, M_TILES], F32, tag=f"acc{i}")
                    nc.vector.tensor_add(nb[:], bacc_t[:], bi[:])
                    bacc_t = nb

            n1 = cpool.tile([128, M_TILES], F32, tag="n1")
            nc.vector.tensor_sub(n1[:], Lh[:], gfull[:])
            nll_t = cpool.tile([128, M_TILES], F32, tag="nll")
            nc.vector.tensor_add(nll_t[:], n1[:], bacc_t[:])
            nc.sync.dma_start(out=out[:], in_=nll_t[:])

    nc.compile()
    return nc


def _to_bf16(x):
    return np.ascontiguousarray(x.astype(ml_dtypes.bfloat16))


def _prep_inputs(hidden, weight, bias, cluster_weight, cluster_bias, target):
    hidden = np.asarray(hidden, np.float32)
    weight = np.asarray(weight, np.float32)
    bias = np.asarray(bias, np.float32)
    cluster_weight = np.asarray(cluster_weight, np.float32)
    cluster_bias = np.asarray(cluster_bias, np.float32)
    tgt = np.asarray(target).astype(np.int64)

    # replicated tensors
    ht = _to_bf16(hidden.reshape(N, K_TILES, 128).transpose(2, 1, 0))
    hb = _to_bf16(hidden.reshape(M_TILES, 128, D).transpose(1, 0, 2))
    gw = _to_bf16(weight[tgt].reshape(M_TILES, 128, D).transpose(1, 0, 2))
    gb = np.ascontiguousarray(bias[tgt].reshape(M_TILES, 128).T)
    cwt = _to_bf16(cluster_weight.reshape(3, K_TILES, 128).transpose(2, 1, 0))
    cbt = np.ascontiguousarray(
        np.broadcast_to(cluster_bias, (128, 3)), dtype=np.float32)
    cl = np.digitize(tgt, CUTOFF_ENDS[1:4])  # 0..3
    mk = np.zeros((N, 3), np.float32)
    for i in (1, 2, 3):
        mk[:, i - 1] = (cl == i)
    mk = np.ascontiguousarray(mk.reshape(M_TILES, 128, 3).transpose(1, 0, 2))

    in_maps = []
    for core in range(N_CORES):
        idx = np.concatenate([
            np.arange(CUTOFF_ENDS[c] + CL_W[c] * core,
                      CUTOFF_ENDS[c] + CL_W[c] * (core + 1))
            for c in range(4)])
        wslice = weight[idx]                              # [12500, 512]
        wt = _to_bf16(wslice.reshape(SLICE, K_TILES, 128).transpose(2, 1, 0))
        ebrow = np.exp(bias[idx].astype(np.float64)).astype(np.float32)
        ebrow[np.isin(idx, CL_ROOT)] = 0.0                # mask root columns
        eb = np.ascontiguousarray(np.broadcast_to(ebrow, (128, SLICE)))
        in_maps.append({
            "wt": wt, "eb": eb, "ht": ht, "hb": hb, "gw": gw, "gb": gb,
            "cw": cwt, "cb": cbt, "mk": mk,
        })
    return in_maps, tgt


def kernel(**inputs) -> np.ndarray:
    in_maps, tgt = _prep_inputs(**inputs)
    if "nc" not in _CACHE:
        _CACHE["nc"] = _build()
    nc = _CACHE["nc"]
    import os
    trace = bool(int(os.environ.get("KERNEL_TRACE", "0")))
    res = bass_utils.run_bass_kernel_spmd(
        nc, in_maps, core_ids=list(range(N_CORES)), trace=trace)
    _CACHE["last_result"] = res
    nll = np.asarray(res.results[0]["nll"])   # [128, 16]
    full = np.ascontiguousarray(nll.T).ravel()  # token t = m*128 + p
    hit = np.isin(tgt, CL_ROOT)
    if hit.any():
        full = full.copy()
        full[hit] = np.inf
    return full.astype(np.float32)
